# revision 1
# baseline (speedup 1.0000x reference)
"""Attention-pooling Trainium2 kernel (8-core SPMD), v9.

Math (matches the jax reference):
    x   = tanh(H @ w1.T); s = x @ w2.T
    S   = segment_softmax(s, batch)   (plain exp - |s|<4, no max-sub)
    out = segment_sum(S * H)

Architecture = the proven v1 pipeline (per-block score2 keeps the s/exp
path entirely on PE/ACT in [slot, block] layout - no cross-queue
relayout chain), plus two upgrades:
  - score-path H ships as float8_e3m4 at 2x scale (w1 pre-halved):
    48.7 MB/core total DMA instead of 65 MB. Simulated rel err 9.1e-3
    (gate 2e-2).
  - accumulation packs 4 blocks per matmul: stationary = 4 blocks'
    one-hot*e weights [128, 32] at col-group g, moving = hg[:, 4 blocks,
    :] [128, 512]; valid results on the block diagonal of each [32, 512]
    psum stripe, garbage elsewhere (ignored by host). 992 -> 248
    matmuls+LDWEIGHTS on the tensor queue.
"""

import os
import numpy as np
import ml_dtypes

D = 128
N_CORES = 8
K = 8              # max segment span per block
CBLK = 32          # blocks per chunk (4096 node slots)
F16 = np.float16
F8 = ml_dtypes.float8_e3m4


# ----------------------------------------------------------------- host prep

def _shard_cuts(batch, n_cores):
    n = batch.shape[0]
    cuts = [0]
    for k in range(1, n_cores):
        t = n * k // n_cores
        cuts.append(int(np.searchsorted(batch, batch[t], side="left")))
    cuts.append(n)
    return cuts


def _greedy_blocks(batch, lo, hi, k_span):
    starts, counts, bases = [], [], []
    i = lo
    while i < hi:
        base = int(batch[i])
        jmax = min(i + 128, hi)
        j = int(np.searchsorted(batch[i:jmax], base + k_span, side="left")) + i
        starts.append(i)
        counts.append(j - i)
        bases.append(base)
        i = j
    return np.array(starts), np.array(counts), np.array(bases)


def _prep_core(H, batch, lo, hi, nblk):
    starts, counts, bases = _greedy_blocks(batch, lo, hi, K)
    nb = len(starts)
    assert nb <= nblk
    nslot = nblk * 128
    slot_node = np.full(nslot, -1, dtype=np.int64)
    for b in range(nb):
        s, c = starts[b], counts[b]
        slot_node[b * 128:b * 128 + c] = np.arange(s, s + c)
    valid = slot_node >= 0

    Hp = np.zeros((nslot, D), dtype=np.float32)
    Hp[valid] = H[slot_node[valid]]
    ht8 = np.ascontiguousarray(
        np.clip(Hp.T * 2.0, -15.5, 15.5)).astype(F8)          # [128, nslot]
    hg = np.ascontiguousarray(
        Hp.astype(F16).reshape(nblk // CBLK, CBLK, 128, D)
        .transpose(0, 2, 1, 3))                               # [nc,128,32,128]

    brel = np.full(nslot, -1.0, dtype=np.float32)
    brel[valid] = (batch[slot_node[valid]]
                   - np.repeat(bases, 128)[: nb * 128][valid[: nb * 128]]
                   ).astype(np.float32)
    brel = np.ascontiguousarray(brel.reshape(nblk, 128).T).astype(F16)

    base_full = np.full(nblk, -1, dtype=np.int64)
    base_full[:nb] = bases
    return dict(ht8=ht8, hg=hg, brel=brel, bases=base_full,
                slot_node=slot_node)


# ------------------------------------------------------------- device kernel

def _build_program(nblk):
    import concourse.bacc as bacc
    import concourse.tile as tile
    from concourse import mybir

    f8 = mybir.dt.float8e3
    f16 = mybir.dt.float16
    f32 = mybir.dt.float32
    nchunk = nblk // CBLK
    CS = CBLK * 128

    nc = bacc.Bacc("TRN2", target_bir_lowering=False, debug=False,
                   num_devices=N_CORES)
    ht_d = nc.dram_tensor("ht8", [D, nblk * 128], f8, kind="ExternalInput")
    hg_d = nc.dram_tensor("hg", [nchunk, D, CBLK, D], f16,
                          kind="ExternalInput")
    brel_d = nc.dram_tensor("brel", [D, nblk], f16, kind="ExternalInput")
    iota_d = nc.dram_tensor("iota", [D, CBLK, K], f16, kind="ExternalInput")
    w1_d = nc.dram_tensor("w1s", [D, D], f16, kind="ExternalInput")
    w2_d = nc.dram_tensor("w2t", [D, 1], f16, kind="ExternalInput")
    num_d = nc.dram_tensor("numout", [nchunk * 2, D, 512], f16,
                           kind="ExternalOutput")
    e_d = nc.dram_tensor("e16o", [D, nblk], f32, kind="ExternalOutput")

    with tile.TileContext(nc) as tc:
        with tc.tile_pool(name="const", bufs=1) as constp, \
             tc.tile_pool(name="ht", bufs=6) as htp, \
             tc.tile_pool(name="hn", bufs=6) as hnp, \
             tc.tile_pool(name="xt", bufs=3) as xtp, \
             tc.tile_pool(name="wm", bufs=6) as wmp, \
             tc.tile_pool(name="nex", bufs=4) as nexp, \
             tc.tile_pool(name="px", bufs=2, space="PSUM") as pxp, \
             tc.tile_pool(name="ps", bufs=2, space="PSUM") as psp, \
             tc.tile_pool(name="pw", bufs=2, space="PSUM") as pwp:

            w1t = constp.tile([D, D], f16)
            nc.gpsimd.dma_start(w1t[:], w1_d.ap())
            w2t = constp.tile([D, 1], f16)
            nc.gpsimd.dma_start(w2t[:], w2_d.ap())
            iotag = constp.tile([D, CBLK, K], f16)
            nc.gpsimd.dma_start(iotag[:], iota_d.ap())
            brel = constp.tile([D, nblk], f16)
            nc.gpsimd.dma_start(brel[:], brel_d.ap())
            ebuf = constp.tile([D, nblk], f32)

            for c in range(nchunk):
                ht = htp.tile([D, CS], f8)
                nc.sync.dma_start(ht[:], ht_d.ap()[:, c * CS:(c + 1) * CS])
                hn = hnp.tile([D, CBLK, D], f16)
                # alternate hn issue across the two HWDGE queues: halves
                # the DMA-issue load on the busy ACT queue
                (nc.sync if c % 2 else nc.scalar).dma_start(
                    hn[:], hg_d.ap()[c])

                xt = xtp.tile([D, CS], f16)
                ps = psp.tile([D, CBLK], f32)
                for j in range(CBLK // 8):
                    px = pxp.tile([D, 1024], f32)
                    for jj in range(2):
                        nc.tensor.matmul(px[:, jj * 512:(jj + 1) * 512],
                                         w1t[:],
                                         ht[:, (2 * j + jj) * 512:(2 * j + jj + 1) * 512],
                                         start=True, stop=True)
                    nc.scalar.activation(xt[:, j * 1024:(j + 1) * 1024],
                                         px[:],
                                         mybir.ActivationFunctionType.Tanh)
                for b in range(CBLK):
                    nc.tensor.matmul(ps[:, b:b + 1],
                                     xt[:, b * 128:(b + 1) * 128],
                                     w2t[:], start=True, stop=True)
                nc.scalar.activation(ebuf[:, c * CBLK:(c + 1) * CBLK],
                                     ps[:],
                                     mybir.ActivationFunctionType.Exp)

                # one-hot x e weights for all CBLK blocks in two DVE ops
                wm = wmp.tile([D, CBLK, K], f16)
                br_b = brel[:, c * CBLK:(c + 1) * CBLK] \
                    .unsqueeze(2).broadcast_to([D, CBLK, K])
                ev_b = ebuf[:, c * CBLK:(c + 1) * CBLK] \
                    .unsqueeze(2).broadcast_to([D, CBLK, K])
                wt = wmp.tile([D, CBLK, K], f16)
                nc.vector.tensor_tensor(wt[:], iotag[:], br_b,
                                        mybir.AluOpType.is_equal)
                nc.vector.tensor_tensor(wm[:], wt[:], ev_b,
                                        mybir.AluOpType.mult)

                # packed accumulation: 4 blocks per matmul, diag valid
                for h in range(2):
                    pw = pwp.tile([D, 512], f32)
                    for g in range(4):
                        t0 = h * 16 + 4 * g
                        nc.tensor.matmul(
                            pw[32 * g:32 * (g + 1), :],
                            wm[:, t0:t0 + 4, :],
                            hn[:, t0:t0 + 4, :],
                            start=True, stop=True,
                            tile_position=(0, 32 * g),
                            skip_group_check=True)
                    nex = nexp.tile([D, 512], f16)
                    nc.vector.tensor_copy(nex[:], pw[:])
                    nc.gpsimd.dma_start(num_d.ap()[2 * c + h], nex[:])

            nc.gpsimd.dma_start(e_d.ap(), ebuf[:])

    nc.compile()
    return nc


# ------------------------------------------------------------------ assembly

def _assemble(size, cores, results):
    num = np.zeros((size, D), dtype=np.float32)
    den = np.zeros(size, dtype=np.float32)
    for core, res in zip(cores, results):
        bases = core["bases"]
        nblk = bases.shape[0]
        # numerator: [ntile, 128, 512]; block t = tile*16 + 4g + i valid at
        # rows 32g+8i+k, cols 128i+f
        no = np.asarray(res["numout"], dtype=np.float32)
        ntile = no.shape[0]
        no = no.reshape(ntile, 4, 4, K, 4, D)     # [tile, g, i, k, b, f]
        i4 = np.arange(4)
        vals = no[:, :, i4, :, i4, :]             # [i, tile, g, k, f]
        vals = np.moveaxis(vals, 0, 2)            # [tile, g, i, k, f]
        vals = np.ascontiguousarray(vals).reshape(nblk * K, D)
        colseg = (np.repeat(bases, K) +
                  np.tile(np.arange(K), nblk))
        ok = np.repeat(bases >= 0, K) & (colseg < size) & (colseg >= 0)
        np.add.at(num, colseg[ok], vals[ok])
        # denominator from exported device e (cast fp16 = device weights)
        e = np.ascontiguousarray(res["e16o"].T).reshape(nblk * 128)
        e = e.astype(np.float16).astype(np.float32)
        sn = core["slot_node"]
        valid = sn >= 0
        np.add.at(den, core["batch_slot"][valid], e[valid])
    return num / (den + 1e-16)[:, None]


# -------------------------------------------------------------------- kernel

def kernel(H, batch, w1, w2, size):
    H = np.asarray(H, dtype=np.float32)
    batch = np.asarray(batch).astype(np.int64)
    w1 = np.asarray(w1, dtype=np.float32)
    w2 = np.asarray(w2, dtype=np.float32)
    size = int(size)
    n = H.shape[0]
    assert H.shape[1] == D

    cuts = _shard_cuts(batch, N_CORES)
    nb_max = 0
    for c in range(N_CORES):
        starts, _, _ = _greedy_blocks(batch, cuts[c], cuts[c + 1], K)
        nb_max = max(nb_max, len(starts))
    nblk = ((nb_max + CBLK - 1) // CBLK) * CBLK

    cores = []
    in_maps = []
    iota = np.broadcast_to(np.arange(K, dtype=F16), (D, CBLK, K)).copy()
    w1s = np.ascontiguousarray(w1.T * 0.5).astype(F16)
    w2t = np.ascontiguousarray(w2.reshape(1, D).T).astype(F16)
    for c in range(N_CORES):
        lo, hi = cuts[c], cuts[c + 1]
        core = _prep_core(H, batch, lo, hi, nblk)
        sn = core["slot_node"]
        core["batch_slot"] = np.where(sn >= 0, batch[np.clip(sn, 0, n - 1)], 0)
        cores.append(core)
        in_maps.append({
            "ht8": core["ht8"], "hg": core["hg"], "brel": core["brel"],
            "iota": iota, "w1s": w1s, "w2t": w2t,
        })

    nc = _build_program(nblk)

    from concourse.bass_utils import run_bass_kernel_spmd
    trace = bool(os.environ.get("ATTN_TRACE"))
    kwargs = {}
    if trace:
        import sys, types
        import antenv
        if "antenv.axon_hooks" not in sys.modules:
            mod = types.ModuleType("antenv.axon_hooks")
            _h = {}
            mod.set_axon_ntff_profile_hook = lambda h: _h.__setitem__("h", h)
            mod.get_axon_ntff_profile_hook = lambda: _h.get("h")
            sys.modules["antenv.axon_hooks"] = mod
            antenv.axon_hooks = mod
        from trn_agent_boot.trn_boot import _ntff_profile_via_ctypes
        sys.modules["antenv.axon_hooks"].set_axon_ntff_profile_hook(
            _ntff_profile_via_ctypes("/opt/axon/libaxon_pjrt.so"))
        from concourse import bass_utils as _bu
        _bu.upload_artifacts = lambda tmpdir: f"local://{tmpdir}"
        tmpdir = os.environ.get("ATTN_TRACE_DIR") or None
        kwargs = dict(trace=True, tmpdir=tmpdir)

    res = run_bass_kernel_spmd(nc, in_maps, list(range(N_CORES)), **kwargs)
    kernel.last_exec_time_ns = res.exec_time_ns
    out = _assemble(size, cores, [res.results[c] for c in range(N_CORES)])
    return out



# revision 2
# speedup vs baseline: 1.2307x; 1.2307x over previous
"""Attention-pooling Trainium2 kernel (8-core SPMD), v10.

Math (matches the jax reference):
    x   = tanh(H @ w1.T); s = x @ w2.T
    S   = segment_softmax(s, batch)   (plain exp - |s|<4, no max-sub)
    out = segment_sum(S * H)

v10 over v9:
  - BOTH H copies ship as float8_e3m4 at 2x scale (score ht8 as before,
    and now the accumulation copy hn8 too): 41.1 MB/core total DMA
    (was 57.8). Host divides the numerator by 2 at assembly. Simulated
    rel err 1.60e-2 (gate 2e-2).
  - e16o exported as f16 (ebuf is f16 end to end).
  - software-pipelined emission: ACT runs tanh back-to-back; each
    chunk's exp/s-matmuls lag one chunk and its accumulation lags two,
    so the strict-FIFO ACT queue never stalls behind PE work (v9 lost
    ~1us/chunk there). PE queue order per step c: score(c), s(c-1),
    accum(c-2). All input DMAs issue on the sync HWDGE ring, keeping
    the ACT queue free of DMA triggers.
"""

import os
import numpy as np
import ml_dtypes

D = 128
N_CORES = 8
K = 8              # max segment span per block
CBLK = 32          # blocks per chunk (4096 node slots)
F16 = np.float16
F8 = ml_dtypes.float8_e3m4


# ----------------------------------------------------------------- host prep

def _shard_cuts(batch, n_cores):
    n = batch.shape[0]
    cuts = [0]
    for k in range(1, n_cores):
        t = n * k // n_cores
        cuts.append(int(np.searchsorted(batch, batch[t], side="left")))
    cuts.append(n)
    return cuts


def _greedy_blocks(batch, lo, hi, k_span):
    starts, counts, bases = [], [], []
    i = lo
    while i < hi:
        base = int(batch[i])
        jmax = min(i + 128, hi)
        j = int(np.searchsorted(batch[i:jmax], base + k_span, side="left")) + i
        starts.append(i)
        counts.append(j - i)
        bases.append(base)
        i = j
    return np.array(starts), np.array(counts), np.array(bases)


def _prep_core(H, batch, lo, hi, nblk):
    starts, counts, bases = _greedy_blocks(batch, lo, hi, K)
    nb = len(starts)
    assert nb <= nblk
    nslot = nblk * 128
    slot_node = np.full(nslot, -1, dtype=np.int64)
    for b in range(nb):
        s, c = starts[b], counts[b]
        slot_node[b * 128:b * 128 + c] = np.arange(s, s + c)
    valid = slot_node >= 0

    Hp = np.zeros((nslot, D), dtype=np.float32)
    Hp[valid] = H[slot_node[valid]]
    h2 = np.clip(Hp * 2.0, -15.5, 15.5)
    ht8 = np.ascontiguousarray(h2.T).astype(F8)              # [128, nslot]
    hn8 = np.ascontiguousarray(
        h2.astype(F8).reshape(nblk // CBLK, CBLK, 128, D)
        .transpose(0, 2, 1, 3))                              # [nc,128,32,128]

    brel = np.full(nslot, -1.0, dtype=np.float32)
    brel[valid] = (batch[slot_node[valid]]
                   - np.repeat(bases, 128)[: nb * 128][valid[: nb * 128]]
                   ).astype(np.float32)
    brel = np.ascontiguousarray(brel.reshape(nblk, 128).T).astype(F16)

    base_full = np.full(nblk, -1, dtype=np.int64)
    base_full[:nb] = bases
    return dict(ht8=ht8, hn8=hn8, brel=brel, bases=base_full,
                slot_node=slot_node)


# ------------------------------------------------------------- device kernel

def _build_program(nblk):
    import concourse.bacc as bacc
    import concourse.tile as tile
    from concourse import mybir

    f8 = mybir.dt.float8e3
    f16 = mybir.dt.float16
    f32 = mybir.dt.float32
    nchunk = nblk // CBLK
    CS = CBLK * 128

    nc = bacc.Bacc("TRN2", target_bir_lowering=False, debug=False,
                   num_devices=N_CORES)
    ht_d = nc.dram_tensor("ht8", [D, nblk * 128], f8, kind="ExternalInput")
    hn_d = nc.dram_tensor("hn8", [nchunk, D, CBLK, D], f8,
                          kind="ExternalInput")
    brel_d = nc.dram_tensor("brel", [D, nblk], f16, kind="ExternalInput")
    iota_d = nc.dram_tensor("iota", [D, CBLK, K], f16, kind="ExternalInput")
    w1_d = nc.dram_tensor("w1s", [D, D], f16, kind="ExternalInput")
    w2_d = nc.dram_tensor("w2t", [D, 1], f16, kind="ExternalInput")
    num_d = nc.dram_tensor("numout", [nchunk * 2, D, 512], f16,
                           kind="ExternalOutput")
    e_d = nc.dram_tensor("e16o", [D, nblk], f16, kind="ExternalOutput")

    with tile.TileContext(nc) as tc:
        with tc.tile_pool(name="const", bufs=1) as constp, \
             tc.tile_pool(name="ht", bufs=8) as htp, \
             tc.tile_pool(name="hn", bufs=8) as hnp, \
             tc.tile_pool(name="xt", bufs=3) as xtp, \
             tc.tile_pool(name="wm", bufs=6) as wmp, \
             tc.tile_pool(name="nex", bufs=4) as nexp, \
             tc.tile_pool(name="px", bufs=2, space="PSUM") as pxp, \
             tc.tile_pool(name="ps", bufs=2, space="PSUM") as psp, \
             tc.tile_pool(name="pw", bufs=2, space="PSUM") as pwp:

            w1t = constp.tile([D, D], f16)
            nc.gpsimd.dma_start(w1t[:], w1_d.ap())
            w2t = constp.tile([D, 1], f16)
            nc.gpsimd.dma_start(w2t[:], w2_d.ap())
            iotag = constp.tile([D, CBLK, K], f16)
            nc.gpsimd.dma_start(iotag[:], iota_d.ap())
            brel = constp.tile([D, nblk], f16)
            nc.gpsimd.dma_start(brel[:], brel_d.ap())
            ebuf = constp.tile([D, nblk], f16)

            hns = {}
            xts = {}
            wms = {}

            def emit_s_exp_wm(c):
                # s-matmuls (xt stationary, w2 moving: big operand rides the
                # LDW port), exp, and one-hot*e weight construction
                xt = xts.pop(c)
                ps = psp.tile([D, CBLK], f32)
                for b in range(CBLK):
                    nc.tensor.matmul(ps[:, b:b + 1],
                                     xt[:, b * 128:(b + 1) * 128],
                                     w2t[:], start=True, stop=True)
                nc.scalar.activation(ebuf[:, c * CBLK:(c + 1) * CBLK],
                                     ps[:],
                                     mybir.ActivationFunctionType.Exp)
                br_b = brel[:, c * CBLK:(c + 1) * CBLK] \
                    .unsqueeze(2).broadcast_to([D, CBLK, K])
                ev_b = ebuf[:, c * CBLK:(c + 1) * CBLK] \
                    .unsqueeze(2).broadcast_to([D, CBLK, K])
                wt = wmp.tile([D, CBLK, K], f16)
                wm = wmp.tile([D, CBLK, K], f16)
                nc.vector.tensor_tensor(wt[:], iotag[:], br_b,
                                        mybir.AluOpType.is_equal)
                nc.vector.tensor_tensor(wm[:], wt[:], ev_b,
                                        mybir.AluOpType.mult)
                wms[c] = wm

            def emit_accum(c):
                # packed accumulation: 4 blocks per matmul, diag valid
                wm = wms.pop(c)
                hn = hns.pop(c)
                for h in range(2):
                    pw = pwp.tile([D, 512], f32)
                    for g in range(4):
                        t0 = h * 16 + 4 * g
                        nc.tensor.matmul(
                            pw[32 * g:32 * (g + 1), :],
                            wm[:, t0:t0 + 4, :],
                            hn[:, t0:t0 + 4, :],
                            start=True, stop=True,
                            tile_position=(0, 32 * g),
                            skip_group_check=True)
                    nex = nexp.tile([D, 512], f16)
                    nc.vector.tensor_copy(nex[:], pw[:])
                    nc.gpsimd.dma_start(num_d.ap()[2 * c + h], nex[:])

            for c in range(nchunk):
                ht = htp.tile([D, CS], f8)
                nc.sync.dma_start(ht[:], ht_d.ap()[:, c * CS:(c + 1) * CS])
                hn = hnp.tile([D, CBLK, D], f8)
                nc.sync.dma_start(hn[:], hn_d.ap()[c])
                hns[c] = hn

                xt = xtp.tile([D, CS], f16)
                xts[c] = xt
                for j in range(CBLK // 8):
                    px = pxp.tile([D, 1024], f32)
                    for jj in range(2):
                        nc.tensor.matmul(px[:, jj * 512:(jj + 1) * 512],
                                         w1t[:],
                                         ht[:, (2 * j + jj) * 512:(2 * j + jj + 1) * 512],
                                         start=True, stop=True)
                    nc.scalar.activation(xt[:, j * 1024:(j + 1) * 1024],
                                         px[:],
                                         mybir.ActivationFunctionType.Tanh)

                if c >= 1:
                    emit_s_exp_wm(c - 1)
                if c >= 2:
                    emit_accum(c - 2)

            emit_s_exp_wm(nchunk - 1)
            emit_accum(nchunk - 2)
            emit_accum(nchunk - 1)

            nc.gpsimd.dma_start(e_d.ap(), ebuf[:])

    nc.compile()
    return nc


# ------------------------------------------------------------------ assembly

def _assemble(size, cores, results):
    num = np.zeros((size, D), dtype=np.float32)
    den = np.zeros(size, dtype=np.float32)
    for core, res in zip(cores, results):
        bases = core["bases"]
        nblk = bases.shape[0]
        # numerator: [ntile, 128, 512]; block t = tile*16 + 4g + i valid at
        # rows 32g+8i+k, cols 128i+f; values are sum(e * 2H) -> halved below
        no = np.asarray(res["numout"], dtype=np.float32)
        ntile = no.shape[0]
        no = no.reshape(ntile, 4, 4, K, 4, D)     # [tile, g, i, k, b, f]
        i4 = np.arange(4)
        vals = no[:, :, i4, :, i4, :]             # [i, tile, g, k, f]
        vals = np.moveaxis(vals, 0, 2)            # [tile, g, i, k, f]
        vals = np.ascontiguousarray(vals).reshape(nblk * K, D)
        colseg = (np.repeat(bases, K) +
                  np.tile(np.arange(K), nblk))
        ok = np.repeat(bases >= 0, K) & (colseg < size) & (colseg >= 0)
        np.add.at(num, colseg[ok], vals[ok])
        # denominator from exported device e (f16 = device weights)
        e = np.ascontiguousarray(res["e16o"].T).reshape(nblk * 128)
        e = e.astype(np.float16).astype(np.float32)
        sn = core["slot_node"]
        valid = sn >= 0
        np.add.at(den, core["batch_slot"][valid], e[valid])
    return (0.5 * num) / (den + 1e-16)[:, None]


# -------------------------------------------------------------------- kernel

def kernel(H, batch, w1, w2, size):
    H = np.asarray(H, dtype=np.float32)
    batch = np.asarray(batch).astype(np.int64)
    w1 = np.asarray(w1, dtype=np.float32)
    w2 = np.asarray(w2, dtype=np.float32)
    size = int(size)
    n = H.shape[0]
    assert H.shape[1] == D

    cuts = _shard_cuts(batch, N_CORES)
    nb_max = 0
    for c in range(N_CORES):
        starts, _, _ = _greedy_blocks(batch, cuts[c], cuts[c + 1], K)
        nb_max = max(nb_max, len(starts))
    nblk = ((nb_max + CBLK - 1) // CBLK) * CBLK

    cores = []
    in_maps = []
    iota = np.broadcast_to(np.arange(K, dtype=F16), (D, CBLK, K)).copy()
    w1s = np.ascontiguousarray(w1.T * 0.5).astype(F16)
    w2t = np.ascontiguousarray(w2.reshape(1, D).T).astype(F16)
    for c in range(N_CORES):
        lo, hi = cuts[c], cuts[c + 1]
        core = _prep_core(H, batch, lo, hi, nblk)
        sn = core["slot_node"]
        core["batch_slot"] = np.where(sn >= 0, batch[np.clip(sn, 0, n - 1)], 0)
        cores.append(core)
        in_maps.append({
            "ht8": core["ht8"], "hn8": core["hn8"], "brel": core["brel"],
            "iota": iota, "w1s": w1s, "w2t": w2t,
        })

    nc = _build_program(nblk)

    from concourse.bass_utils import run_bass_kernel_spmd
    trace = bool(os.environ.get("ATTN_TRACE"))
    kwargs = {}
    if trace:
        import sys, types
        import antenv
        if "antenv.axon_hooks" not in sys.modules:
            mod = types.ModuleType("antenv.axon_hooks")
            _h = {}
            mod.set_axon_ntff_profile_hook = lambda h: _h.__setitem__("h", h)
            mod.get_axon_ntff_profile_hook = lambda: _h.get("h")
            sys.modules["antenv.axon_hooks"] = mod
            antenv.axon_hooks = mod
        from trn_agent_boot.trn_boot import _ntff_profile_via_ctypes
        sys.modules["antenv.axon_hooks"].set_axon_ntff_profile_hook(
            _ntff_profile_via_ctypes("/opt/axon/libaxon_pjrt.so"))
        from concourse import bass_utils as _bu
        _bu.upload_artifacts = lambda tmpdir: f"local://{tmpdir}"
        tmpdir = os.environ.get("ATTN_TRACE_DIR") or None
        kwargs = dict(trace=True, tmpdir=tmpdir)

    res = run_bass_kernel_spmd(nc, in_maps, list(range(N_CORES)), **kwargs)
    kernel.last_exec_time_ns = res.exec_time_ns
    out = _assemble(size, cores, [res.results[c] for c in range(N_CORES)])
    return out
